# revision 10
# baseline (speedup 1.0000x reference)
"""Self-contained 2-layer GAT kernel for Trainium2 (8 NeuronCores, SPMD).

Strategy (edge-parallel by destination):
  - Nodes padded to 50176 = 392 windows of 128; core k owns 49 windows.
  - Edges (incl. self-loops) are assigned to the core owning their dst window.
  - Per core: projection of the full node table (replicated), then per window
    gather source rows (dma_gather spread over the 4 SWDGE queues so all four
    Q7 core-pairs generate descriptors in parallel), compute edge softmax
    numerators, scatter-add via one-hot matmuls accumulated in PSUM, normalize.
  - Layer-2 node table is built per-slice and exchanged with an AllGather.
"""
import numpy as np

import concourse.bass as bass
import concourse.mybir as mybir
import concourse.tile as tile
from concourse import bacc

F16 = mybir.dt.float16
F32 = mybir.dt.float32
I16 = mybir.dt.int16
AF = mybir.ActivationFunctionType
OP = mybir.AluOpType

N = 50000
NPAD = 50176          # 392 * 128
NCORES = 8
WPC = 49              # windows per core
SLICE = NPAD // NCORES  # 6272
HALF = 32768          # int16 gather index cutoff
GW = 3                # windows per edge-phase group
PSUP = 8              # projection tiles per super-tile
NEG_SLOPE = 0.2
NQ = 4                # SWDGE queues


# ---------------------------------------------------------------- gather op
def _dma_gather_raw(nc, out_ap, in_ap, idxs_ap, num_idxs, elem_size, elem_step,
                    queue_num=0):
    """nc.gpsimd.dma_gather without the elem_size%256 restriction
    (non-transpose DRAM->SBUF path only; elem_step bytes must be %256)."""
    from concourse._compat import exact_div
    eng = nc.gpsimd
    assert idxs_ap.dtype == I16
    assert in_ap.space == bass.MemorySpace.DRAM
    assert out_ap.space == bass.MemorySpace.SBUF
    assert in_ap.ap[-1][1] == elem_size
    assert in_ap.ap[0][0] == elem_step
    stride_bytes = elem_step * mybir.dt.size(in_ap.dtype)
    stride_bytes_256 = exact_div(stride_bytes, 256)
    assert stride_bytes_256 < 256
    _in_ap = eng.lower_ap_dma(in_ap, for_custom_bir_dma=True)
    _idxs_ap = eng.lower_ap(idxs_ap)
    _out_ap = eng.lower_ap(out_ap)
    return eng.add_instruction(
        mybir.InstDMAGatherAnt(
            name=nc.get_next_instruction_name(),
            ins=[*_in_ap, _idxs_ap, eng.lower_val_access(eng.to_reg(num_idxs))],
            outs=[_out_ap],
            transpose=False,
            num_idxs=num_idxs,
            elem_size=elem_size,
            stride_bytes_256=stride_bytes_256,
            gen_mode=0,
            single_packet=False,
            queue_num=queue_num,
            sbuf_tokens_per_rank=0,
            sbuf_free_dim_per_rank=0,
            sbuf_free_dim_pad_per_rank=0,
            sbuf_byte_offset=0,
        )
    )


def _bc(ap, dims):
    """Return copy of AP with free dims replaced by `dims` ([step, count] list)."""
    return bass.AP(ap.tensor, ap.offset, [ap.ap[0]] + dims)


# ---------------------------------------------------------------- host prep
def _build_plan(src, dst):
    """Static plan + per-core metadata arrays. src/dst int64 incl self-loops."""
    E = len(src)
    stream = (src >= HALF).astype(np.int64)
    win = (dst >> 7).astype(np.int64)
    order = np.lexsort((stream, win))
    s_src = src[order]
    s_dst = dst[order]
    s_str = stream[order]
    s_win = win[order]
    key = s_win * 2 + s_str
    cnt = np.bincount(key, minlength=392 * 2).reshape(392, 2)
    kslot = -(-cnt.reshape(NCORES, WPC, 2).max(axis=0) // 128)  # [WPC, 2]

    gdefs = [list(range(i, min(i + GW, WPC))) for i in range(0, WPC, GW)]
    groups = []
    totblk = la = lb = lt = 0
    qc = 0  # round-robin queue counter
    colbase = np.zeros((WPC, 2), np.int64)
    for gws in gdefs:
        ka_g = int(kslot[gws, 0].sum())
        kb_g = int(kslot[gws, 1].sum())
        nb_g = ka_g + kb_g
        wins = []
        aoff = boff = 0
        for w in gws:
            ka, kb = int(kslot[w, 0]), int(kslot[w, 1])
            colbase[w, 0] = totblk + aoff
            colbase[w, 1] = totblk + ka_g + boff
            wins.append(dict(w=w, ka=ka, kb=kb,
                             acols=list(range(totblk + aoff, totblk + aoff + ka)),
                             bcols=list(range(totblk + ka_g + boff,
                                              totblk + ka_g + boff + kb))))
            aoff += ka
            boff += kb
        # gather pieces: split A and AD in two, rotate queues round-robin
        pieces = []
        for kind, b0, b1 in (
            ("AD", 0, nb_g // 2), ("A", 0, ka_g // 2),
            ("AD", nb_g // 2, nb_g), ("A", ka_g // 2, ka_g),
            ("B", 0, kb_g),
        ):
            if b1 > b0:
                pieces.append((kind, b0, b1, qc % NQ))
                qc += 1
        groups.append(dict(cb=totblk, ka=ka_g, kb=kb_g, wins=wins,
                           a16=la, b16=lb, t16=lt, pieces=pieces))
        totblk += nb_g
        la += ka_g * 8
        lb += kb_g * 8
        lt += nb_g * 8
    plan = dict(groups=groups, totblk=totblk, la16=la, lb16=lb, lt16=lt)

    # per-edge placement
    run_start = np.searchsorted(key, np.arange(392 * 2), side="left")
    rank = np.arange(E) - run_start[key]
    blk = rank >> 7
    row = rank & 127
    core = s_win // WPC
    wslot = s_win % WPC
    col = colbase[wslot, s_str] + blk  # global block column [0, totblk)

    # flat gather positions
    cb_of = np.zeros(WPC, np.int64)
    ka_of = np.zeros(WPC, np.int64)
    aoffe = np.zeros(WPC, np.int64)   # edge offset of group's A region
    boffe = np.zeros(WPC, np.int64)
    toffe = np.zeros(WPC, np.int64)
    for g in groups:
        for wi in g["wins"]:
            w = wi["w"]
            cb_of[w] = g["cb"]
            ka_of[w] = g["ka"]
            aoffe[w] = g["a16"] * 16
            boffe[w] = g["b16"] * 16
            toffe[w] = g["t16"] * 16
    rel = col - cb_of[wslot]
    t_a = aoffe[wslot] + rel * 128 + row                    # stream A only
    t_b = boffe[wslot] + (rel - ka_of[wslot]) * 128 + row   # stream B only
    t_t = toffe[wslot] + rel * 128 + row                    # all edges

    def wrap(flat):
        w16 = flat.reshape(-1, 16).T.astype(np.int16)       # [16, L/16]
        return np.tile(w16, (8, 1))                         # [128, L/16]

    metas = []
    for c in range(NCORES):
        m = core == c
        dl = np.full((128, plan["totblk"]), -1.0, np.float16)
        dl[row[m], col[m]] = (s_dst[m] - (c * SLICE + wslot[m] * 128)
                              ).astype(np.float16)
        fa = np.zeros(la * 16, np.int64)
        mA = m & (s_str == 0)
        fa[t_a[mA]] = s_src[mA]
        fb = np.zeros(lb * 16, np.int64)
        mB = m & (s_str == 1)
        fb[t_b[mB]] = s_src[mB] - HALF
        ft = np.zeros(lt * 16, np.int64)
        ft[t_t[m]] = s_dst[m] - c * SLICE
        metas.append(dict(meta_dl=dl, meta_a=wrap(fa), meta_b=wrap(fb),
                          meta_ad=wrap(ft)))
    return plan, metas


def _pack_weights(W1, as1, ad1, b1, W2, as2, ad2, b2):
    """Host packing with (c-major, head-minor) column interleave for layer 1."""
    H, CH = as1.shape  # 4, 32
    perm = np.array([hd * CH + c for c in range(CH) for hd in range(H)])
    W1p = W1[:, perm]                                   # [128, 128]
    As1 = np.zeros((128, H), np.float64)
    Ad1 = np.zeros((128, H), np.float64)
    for c in range(CH):
        for hd in range(H):
            As1[c * H + hd, hd] = as1[hd, c]
            Ad1[c * H + hd, hd] = ad1[hd, c]
    W1cat = np.concatenate([W1p, W1p @ As1, W1p @ Ad1], 1
                           ).astype(np.float16)   # [128,136]
    W2p = W2[perm, :]                                   # [128, 64]
    As2 = W2p @ as2[0]
    Ad2 = W2p @ ad2[0]
    W2cat = np.concatenate([W2p, As2[:, None], Ad2[:, None]], 1
                           ).astype(np.float16)          # [128, 66]
    b1rep = np.tile(b1[perm].astype(np.float32), (128, 1))   # [128,128]
    b2rep = np.tile(b2.astype(np.float32), (128, 1))         # [128, 64]
    return W1cat, W2cat, b1rep, b2rep


# ---------------------------------------------------------------- program
def _build_program(plan):
    nc = bacc.Bacc(None, target_bir_lowering=False, num_swdge_queues=NQ)
    totblk = plan["totblk"]
    la16, lb16, lt16 = plan["la16"], plan["lb16"], plan["lt16"]

    xT = nc.declare_dram_parameter("xT", [128, NPAD], F16, isOutput=False)
    xTm = nc.declare_dram_parameter("xTm", [128, SLICE], F16, isOutput=False)
    W1cat = nc.declare_dram_parameter("W1cat", [128, 136], F16, isOutput=False)
    W2cat = nc.declare_dram_parameter("W2cat", [128, 66], F16, isOutput=False)
    b1rep = nc.declare_dram_parameter("b1rep", [128, 128], F32, isOutput=False)
    b2rep = nc.declare_dram_parameter("b2rep", [128, 64], F32, isOutput=False)
    iota = nc.declare_dram_parameter("iota", [128, 128], F16, isOutput=False)
    iden = nc.declare_dram_parameter("iden", [128, 128], F16, isOutput=False)
    meta_dl = nc.declare_dram_parameter("meta_dl", [128, totblk], F16, isOutput=False)
    meta_a = nc.declare_dram_parameter("meta_a", [128, la16], I16, isOutput=False)
    meta_b = nc.declare_dram_parameter("meta_b", [128, lb16], I16, isOutput=False)
    meta_ad = nc.declare_dram_parameter("meta_ad", [128, lt16], I16, isOutput=False)
    out = nc.declare_dram_parameter("out", [SLICE, 64], F32, isOutput=True)

    table1 = nc.dram_tensor("table1", [NPAD, 256], F16)    # [h(128i), a_s(4)]
    adloc1 = nc.dram_tensor("adloc1", [SLICE, 128], F16)   # a_d(4) local slice
    h1 = nc.dram_tensor("h1", [SLICE, 128], F16)           # layer-1 out slice
    t2own = nc.dram_tensor("t2own", [SLICE, 128], F16)     # [h2(64),as2,ad2]
    t2full = nc.dram_tensor("t2full", [NPAD, 128], F16, addr_space="Shared")

    with tile.TileContext(nc) as tc:
        with (
            tc.tile_pool(name="const", bufs=1) as cp,
            tc.tile_pool(name="meta", bufs=1) as mp,
            tc.tile_pool(name="proj", bufs=3) as pp,
            tc.tile_pool(name="projps", bufs=3, space="PSUM") as pps,
            tc.tile_pool(name="edge", bufs=3) as ep,
            tc.tile_pool(name="oh", bufs=8) as ohp,
            tc.tile_pool(name="edgeps", bufs=5, space="PSUM") as eps,
            tc.tile_pool(name="post", bufs=6) as qp,
        ):
            # ---- persistent constants / metadata
            w1_sb = cp.tile([128, 136], F16)
            nc.sync.dma_start(out=w1_sb[:], in_=W1cat[:])
            w2_sb = cp.tile([128, 66], F16)
            nc.sync.dma_start(out=w2_sb[:], in_=W2cat[:])
            b1_sb = cp.tile([128, 128], F32)
            nc.sync.dma_start(out=b1_sb[:], in_=b1rep[:])
            b2_sb = cp.tile([128, 64], F32)
            nc.sync.dma_start(out=b2_sb[:], in_=b2rep[:])
            iota_sb = cp.tile([128, 128], F16)
            nc.sync.dma_start(out=iota_sb[:], in_=iota[:])
            iden_sb = cp.tile([128, 128], F16)
            nc.sync.dma_start(out=iden_sb[:], in_=iden[:])
            dl_sb = mp.tile([128, totblk], F16)
            nc.scalar.dma_start(out=dl_sb[:], in_=meta_dl[:])
            ia_sb = mp.tile([128, la16], I16)
            nc.scalar.dma_start(out=ia_sb[:], in_=meta_a[:])
            ib_sb = mp.tile([128, lb16], I16)
            nc.scalar.dma_start(out=ib_sb[:], in_=meta_b[:])
            it_sb = mp.tile([128, lt16], I16)
            nc.scalar.dma_start(out=it_sb[:], in_=meta_ad[:])

            # ---- P1: full projection -> table1
            for s in range(NPAD // (128 * PSUP)):           # 49 super-tiles
                xt = pp.tile([128, 128 * PSUP], F16)
                nc.sync.dma_start(
                    out=xt[:], in_=xT[:, s * 128 * PSUP:(s + 1) * 128 * PSUP])
                rows = pp.tile([128, PSUP, 136], F16)
                for j in range(PSUP):
                    ps = pps.tile([128, 136], F32, tag="pp")
                    nc.tensor.matmul(out=ps[:], lhsT=xt[:, j * 128:(j + 1) * 128],
                                     rhs=w1_sb[:], start=True, stop=True)
                    if j % 2 == 0:
                        nc.scalar.copy(out=rows[:, j, :], in_=ps[:])
                    else:
                        nc.vector.tensor_copy(out=rows[:, j, :], in_=ps[:])
                dst = table1[s * 128 * PSUP:(s + 1) * 128 * PSUP, 0:136]
                dst = dst.rearrange("(j p) c -> p j c", p=128)
                nc.gpsimd.dma_start(out=dst, in_=rows[:, :, :])

            # ---- MINI: own-slice a_d -> adloc1
            nsup = -(-WPC // PSUP)
            for s in range(nsup):
                w0 = s * PSUP
                nw = min(PSUP, WPC - w0)
                xt = pp.tile([128, 128 * PSUP], F16)
                nc.sync.dma_start(
                    out=xt[:, 0:128 * nw],
                    in_=xTm[:, w0 * 128:(w0 + nw) * 128])
                rows = pp.tile([128, PSUP, 4], F16)
                for j in range(nw):
                    ps = pps.tile([128, 136], F32, tag="pp")
                    nc.tensor.matmul(out=ps[:, 0:4], lhsT=xt[:, j * 128:(j + 1) * 128],
                                     rhs=w1_sb[:, 132:136], start=True, stop=True)
                    if j % 2 == 0:
                        nc.scalar.copy(out=rows[:, j, :], in_=ps[:, 0:4])
                    else:
                        nc.vector.tensor_copy(out=rows[:, j, :], in_=ps[:, 0:4])
                dst = adloc1[w0 * 128:(w0 + nw) * 128, 0:4]
                dst = dst.rearrange("(j p) c -> p j c", p=128)
                nc.sync.dma_start(out=dst, in_=rows[:, 0:nw, :])

            tc.strict_bb_all_engine_barrier()

            # ---- edge phase (shared for both layers)
            def edge_phase(layer):
                if layer == 1:
                    elem, adw, hc, rw = 132, 4, 128, 132
                    tblA = table1[0:HALF, 0:elem]
                    tblB = table1[HALF:NPAD, 0:elem]
                    adap = adloc1[:, 0:adw]
                    estep, astep = 256, 128
                else:
                    elem, adw, hc, rw = 66, 1, 64, 65
                    tblA = t2full[0:HALF, 0:elem]
                    tblB = t2full[HALF:NPAD, 0:elem]
                    adap = t2own[:, 65:66]
                    estep, astep = 128, 128

                def emit_front(g):
                    """Gathers (Pool) + one-hots (DVE) + per-edge prep."""
                    ka, kb, cb = g["ka"], g["kb"], g["cb"]
                    nb = ka + kb
                    G = ep.tile([128, nb, elem], F16, tag="G")
                    AD = ep.tile([128, nb, adw], F16, tag="AD")
                    for kind, b0, b1, q in g["pieces"]:
                        nn = (b1 - b0) * 128
                        if kind == "A":
                            _dma_gather_raw(
                                nc, G[:, b0:b1, :], tblA,
                                ia_sb[:, g["a16"] + b0 * 8:g["a16"] + b1 * 8],
                                nn, elem, estep, q)
                        elif kind == "B":
                            _dma_gather_raw(
                                nc, G[:, ka + b0:ka + b1, :], tblB,
                                ib_sb[:, g["b16"] + b0 * 8:g["b16"] + b1 * 8],
                                nn, elem, estep, q)
                        else:
                            _dma_gather_raw(
                                nc, AD[:, b0:b1, :], adap,
                                it_sb[:, g["t16"] + b0 * 8:g["t16"] + b1 * 8],
                                nn, adw, astep, q)
                    # one-hots first: no data deps, keeps PE fed
                    ohs = []
                    for wi in g["wins"]:
                        nbw = wi["ka"] + wi["kb"]
                        OHt = ohp.tile([128, nbw, 128], F16, tag="oh")
                        for off, cl in ((0, wi["acols"]), (wi["ka"], wi["bcols"])):
                            if not cl:
                                continue
                            n = len(cl)
                            nc.vector.tensor_tensor(
                                out=OHt[:, off:off + n, :],
                                in0=_bc(iota_sb[:, 0:1], [[0, n], [1, 128]]),
                                in1=_bc(dl_sb[:, cl[0]:cl[0] + 1],
                                        [[1, n], [0, 128]]),
                                op=OP.is_equal)
                        ohs.append(OHt)
                    # ex = exp(leaky(a_s + a_d)), batched over the group
                    LG = ep.tile([128, nb, adw], F16, tag="LG")
                    nc.vector.tensor_tensor(
                        out=LG[:, :, :], in0=G[:, :, hc:hc + adw],
                        in1=AD[:, :, :], op=OP.add)
                    T1 = ep.tile([128, nb, adw], F16, tag="T1")
                    nc.vector.tensor_scalar(
                        out=T1[:, :, :], in0=LG[:, :, :],
                        scalar1=NEG_SLOPE, scalar2=None, op0=OP.mult)
                    nc.vector.tensor_tensor(
                        out=T1[:, :, :], in0=LG[:, :, :], in1=T1[:, :, :],
                        op=OP.max)
                    EX = ep.tile([128, nb, adw], F16, tag="EX")
                    nc.scalar.activation(out=EX[:, :, :], in_=T1[:, :, :],
                                         func=AF.Exp)
                    # rhs = [ex | ex * h]
                    RHS = ep.tile([128, nb, rw], F16, tag="R")
                    nc.scalar.copy(out=RHS[:, :, 0:adw], in_=EX[:, :, :])
                    if layer == 1:
                        g_h = G[:, :, 0:hc].rearrange("p b (c h) -> p b c h", h=4)
                        r_h = RHS[:, :, adw:rw].rearrange(
                            "p b (c h) -> p b c h", h=4)
                        exb = _bc(EX[:, :, :], [[4, nb], [0, 32], [1, 4]])
                        nc.vector.tensor_tensor(out=r_h, in0=g_h, in1=exb,
                                                op=OP.mult)
                    else:
                        exb = _bc(EX[:, :, :], [[1, nb], [0, hc]])
                        nc.vector.tensor_tensor(
                            out=RHS[:, :, 1:rw], in0=G[:, :, 0:hc], in1=exb,
                            op=OP.mult)
                    return dict(RHS=RHS, ohs=ohs)

                def emit_back(g, st):
                    """Scatter matmuls (PE), then normalize/activation + P2."""
                    cb = g["cb"]
                    RHS, ohs = st["RHS"], st["ohs"]
                    pss = []
                    for wi, OHt in zip(g["wins"], ohs):
                        cols = wi["acols"] + wi["bcols"]
                        ps = eps.tile([128, 132], F32, tag="eps")
                        for j, c_ in enumerate(cols):
                            nc.tensor.matmul(
                                out=ps[:, 0:rw], lhsT=OHt[:, j, :],
                                rhs=RHS[:, c_ - cb, :],
                                start=(j == 0), stop=(j == len(cols) - 1))
                        pss.append(ps)
                    for wi, ps in zip(g["wins"], pss):
                        w = wi["w"]
                        rc = qp.tile([128, adw], F32, tag="rc")
                        nc.vector.reciprocal(rc[:], ps[:, 0:adw])
                        z = qp.tile([128, hc], F32, tag="z")
                        if layer == 1:
                            z_v = z[:].rearrange("p (c h) -> p c h", h=4)
                            p_v = ps[:, adw:rw].rearrange("p (c h) -> p c h", h=4)
                            rcb = _bc(rc[:], [[0, 32], [1, 4]])
                        else:
                            z_v = z[:]
                            p_v = ps[:, adw:rw]
                            rcb = _bc(rc[:], [[0, 64]])
                        nc.vector.tensor_tensor(out=z_v, in0=p_v, in1=rcb,
                                                op=OP.mult)
                        bias = b1_sb if layer == 1 else b2_sb
                        nc.vector.tensor_tensor(out=z[:], in0=z[:], in1=bias[:],
                                                op=OP.add)
                        if layer == 1:
                            m = qp.tile([128, hc], F32, tag="m")
                            nc.vector.tensor_scalar(
                                out=m[:], in0=z[:], scalar1=0.0, scalar2=None,
                                op0=OP.min)
                            e = qp.tile([128, hc], F32, tag="e")
                            nc.scalar.activation(out=e[:], in_=m[:], func=AF.Exp)
                            r = qp.tile([128, hc], F32, tag="r")
                            nc.vector.tensor_scalar(
                                out=r[:], in0=z[:], scalar1=0.0, scalar2=-1.0,
                                op0=OP.max, op1=OP.add)
                            hp = qp.tile([128, hc], F16, tag="hp")
                            nc.vector.tensor_tensor(out=hp[:], in0=e[:],
                                                    in1=r[:], op=OP.add)
                            nc.sync.dma_start(
                                out=h1[w * 128:(w + 1) * 128, :], in_=hp[:])
                        else:
                            nc.scalar.dma_start(
                                out=out[w * 128:(w + 1) * 128, :], in_=z[:])

                groups = plan["groups"]
                st = None
                for gi in range(len(groups) + 1):
                    nst = emit_front(groups[gi]) if gi < len(groups) else None
                    if gi >= 1:
                        emit_back(groups[gi - 1], st)
                    st = nst

            edge_phase(1)
            tc.strict_bb_all_engine_barrier()

            # ---- P2: own-slice projection -> t2own
            for s2 in range(-(-WPC // 4)):
                w0 = s2 * 4
                nw = min(4, WPC - w0)
                lt = pp.tile([128, 512], F16, tag="lt")
                nc.sync.dma_start_transpose(
                    out=lt[:, 0:nw * 128],
                    in_=h1[w0 * 128:(w0 + nw) * 128, :])
                rows = pp.tile([128, 4, 66], F16, tag="rows2")
                for j in range(nw):
                    ps = pps.tile([128, 136], F32, tag="pp")
                    nc.tensor.matmul(out=ps[:, 0:66],
                                     lhsT=lt[:, j * 128:(j + 1) * 128],
                                     rhs=w2_sb[:], start=True, stop=True)
                    if j % 2 == 0:
                        nc.scalar.copy(out=rows[:, j, :], in_=ps[:, 0:66])
                    else:
                        nc.vector.tensor_copy(out=rows[:, j, :], in_=ps[:, 0:66])
                dst = t2own[w0 * 128:(w0 + nw) * 128, 0:66]
                dst = dst.rearrange("(j p) c -> p j c", p=128)
                nc.sync.dma_start(out=dst, in_=rows[:, 0:nw, :])

            tc.strict_bb_all_engine_barrier()
            nc.gpsimd.collective_compute(
                "AllGather", OP.bypass,
                replica_groups=[list(range(NCORES))],
                ins=[t2own[:]], outs=[t2full[:]])
            tc.strict_bb_all_engine_barrier()
            edge_phase(2)

    nc.finalize()
    return nc


# ---------------------------------------------------------------- runner
def _make_runner(nc, n_cores):
    import jax
    from jax.sharding import Mesh, PartitionSpec, NamedSharding
    from jax.experimental.shard_map import shard_map
    from concourse.bass2jax import (_bass_exec_p, partition_id_tensor,
                                    install_neuronx_cc_hook)

    install_neuronx_cc_hook()
    partition_name = nc.partition_id_tensor.name if nc.partition_id_tensor else None
    in_names, out_names, out_avals, zero_outs = [], [], [], []
    for alloc in nc.m.functions[0].allocations:
        if not isinstance(alloc, mybir.MemoryLocationSet):
            continue
        name = alloc.memorylocations[0].name
        if alloc.kind == "ExternalInput":
            if name != partition_name:
                in_names.append(name)
        elif alloc.kind == "ExternalOutput":
            out_names.append(name)
            shape = tuple(alloc.tensor_shape)
            dtype = mybir.dt.np(alloc.dtype)
            out_avals.append(jax.core.ShapedArray(shape, dtype))
            zero_outs.append(np.zeros(shape, dtype))

    n_params = len(in_names)
    n_outs = len(out_avals)
    all_in = list(in_names) + list(out_names)
    if partition_name is not None:
        all_in.append(partition_name)

    def _body(*args):
        operands = list(args)
        if partition_name is not None:
            operands.append(partition_id_tensor())
        outs = _bass_exec_p.bind(
            *operands, out_avals=tuple(out_avals), in_names=tuple(all_in),
            out_names=tuple(out_names), lowering_input_output_aliases=(),
            sim_require_finite=False, sim_require_nnan=False, nc=nc)
        return tuple(outs)

    devices = jax.devices()[:n_cores]
    mesh = Mesh(np.asarray(devices), ("core",))
    specs = (PartitionSpec("core"),) * (n_params + n_outs)
    sharded = jax.jit(
        shard_map(_body, mesh=mesh, in_specs=specs,
                  out_specs=(PartitionSpec("core"),) * n_outs, check_rep=False),
        keep_unused=True)

    shard = NamedSharding(mesh, PartitionSpec("core"))
    devcache = {}

    def run(in_maps):
        import hashlib
        hsh = hashlib.md5()
        for m in in_maps:
            for nm in in_names:
                hsh.update(np.asarray(m[nm]).tobytes())
        key = hsh.hexdigest()
        dev = devcache.get(key)
        if dev is None:
            per_core = [[np.asarray(m[nm]) for nm in in_names] for m in in_maps]
            concat_in = [
                np.concatenate([per_core[c][i] for c in range(n_cores)], 0)
                for i in range(n_params)]
            concat_zero = [
                np.zeros((n_cores * z.shape[0], *z.shape[1:]), z.dtype)
                for z in zero_outs]
            dev = ([jax.device_put(a, shard) for a in concat_in],
                   [jax.device_put(z, shard) for z in concat_zero])
            jax.block_until_ready(dev)
            devcache.clear()
            devcache[key] = dev
        dev_in, dev_zero = dev
        outs = sharded(*dev_in, *dev_zero)
        jax.block_until_ready(outs)
        return [
            {nm: np.asarray(outs[i]).reshape(n_cores, *out_avals[i].shape)[c]
             for i, nm in enumerate(out_names)}
            for c in range(n_cores)
        ]

    return run


_CACHE = {}
_LAST_NC = [None]


def last_nc():
    return _LAST_NC[0]


def kernel(x, edge_index, W1, att_src1, att_dst1, b1, W2, att_src2, att_dst2, b2):
    x = np.asarray(x)
    edge_index = np.asarray(edge_index)
    src = np.concatenate([edge_index[0], np.arange(N, dtype=np.int64)])
    dst = np.concatenate([edge_index[1], np.arange(N, dtype=np.int64)])

    ck = hash((src.tobytes(), dst.tobytes()))
    if ck in _CACHE:
        plan, metas, run = _CACHE[ck]
    else:
        plan, metas = _build_plan(src, dst)
        nc = _build_program(plan)
        run = _make_runner(nc, NCORES)
        _CACHE[ck] = (plan, metas, run)
        _LAST_NC[0] = nc

    W1cat, W2cat, b1rep, b2rep = _pack_weights(
        np.asarray(W1, np.float64), np.asarray(att_src1, np.float64),
        np.asarray(att_dst1, np.float64), np.asarray(b1, np.float64),
        np.asarray(W2, np.float64), np.asarray(att_src2, np.float64),
        np.asarray(att_dst2, np.float64), np.asarray(b2, np.float64))
    xT = np.zeros((128, NPAD), np.float16)
    xT[:, :N] = np.asarray(x, np.float32).T.astype(np.float16)
    iota = np.tile(np.arange(128, dtype=np.float16), (128, 1))
    iden = np.eye(128, dtype=np.float16)

    in_maps = []
    for c in range(NCORES):
        m = dict(metas[c])
        m.update(xT=xT, xTm=np.ascontiguousarray(xT[:, c * SLICE:(c + 1) * SLICE]),
                 W1cat=W1cat, W2cat=W2cat, b1rep=b1rep, b2rep=b2rep, iota=iota,
                 iden=iden)
        in_maps.append(m)

    res = run(in_maps)
    full = np.concatenate([res[c]["out"] for c in range(NCORES)], 0)
    return full[:N].astype(np.float32)



# revision 11
# speedup vs baseline: 325.3421x; 325.3421x over previous
"""Self-contained 2-layer GAT kernel for Trainium2 (8 NeuronCores, SPMD).

Strategy (edge-parallel by destination, CSR-hybrid):
  - Nodes padded to 50176 = 392 windows of 128; core k owns 49 windows.
  - Projections are data-parallel: each core projects its own 6272-node
    slice, then an AllGather replicates the node table.
  - Edges with dst in window w are stored CSR-style: slot s of dst-row p
    holds the s-th incoming edge of node p.  The gather lands
    [dst-row(partition), slot, feature], so the softmax denominator and
    the weighted aggregation are free-dim reduces, and a_d[dst] is a
    partition-broadcast (no per-edge a_d gather for the dense part).
  - Per (window, stream) the first S0 edges per dst are dense CSR slots;
    the remainder ("overflow") goes through one-hot scatter matmuls.
    Streams A/B split src at 32768 to fit int16 gather indices.
  - Self-loop rows arrive via plain DMAs (no gather descriptors).
  - Layer 2 reuses the same index metadata (same graph, same layout).
"""
import numpy as np

import concourse.bass as bass
import concourse.mybir as mybir
import concourse.tile as tile
from concourse import bacc

F16 = mybir.dt.float16
F32 = mybir.dt.float32
I16 = mybir.dt.int16
AF = mybir.ActivationFunctionType
OP = mybir.AluOpType

N = 50000
NPAD = 50176          # 392 * 128
NCORES = 8
WPC = 49              # windows per core
SLICE = NPAD // NCORES  # 6272
HALF = 32768          # int16 gather index cutoff
GW = 7                # windows per edge-phase group
NGRP = WPC // GW      # 7
S0A = 10              # dense CSR slots, stream A (src < HALF)
S0B = 6               # dense CSR slots, stream B
SD = 1 + S0A + S0B    # dense slots incl self-loop slot
NBD = GW * SD         # dense blocks per group
NEG_SLOPE = 0.2
NQ = 4                # SWDGE queues


# ---------------------------------------------------------------- gather op
def _dma_gather_raw(nc, out_ap, in_ap, idxs_ap, num_idxs, elem_size, elem_step,
                    queue_num=0):
    """nc.gpsimd.dma_gather without the elem_size%256 restriction
    (non-transpose DRAM->SBUF path only; elem_step bytes must be %256)."""
    from concourse._compat import exact_div
    eng = nc.gpsimd
    assert idxs_ap.dtype == I16
    assert in_ap.space == bass.MemorySpace.DRAM
    assert out_ap.space == bass.MemorySpace.SBUF
    assert in_ap.ap[-1][1] == elem_size
    assert in_ap.ap[0][0] == elem_step
    stride_bytes = elem_step * mybir.dt.size(in_ap.dtype)
    stride_bytes_256 = exact_div(stride_bytes, 256)
    assert stride_bytes_256 < 256
    _in_ap = eng.lower_ap_dma(in_ap, for_custom_bir_dma=True)
    _idxs_ap = eng.lower_ap(idxs_ap)
    _out_ap = eng.lower_ap(out_ap)
    return eng.add_instruction(
        mybir.InstDMAGatherAnt(
            name=nc.get_next_instruction_name(),
            ins=[*_in_ap, _idxs_ap, eng.lower_val_access(eng.to_reg(num_idxs))],
            outs=[_out_ap],
            transpose=False,
            num_idxs=num_idxs,
            elem_size=elem_size,
            stride_bytes_256=stride_bytes_256,
            gen_mode=0,
            single_packet=False,
            queue_num=queue_num,
            sbuf_tokens_per_rank=0,
            sbuf_free_dim_per_rank=0,
            sbuf_free_dim_pad_per_rank=0,
            sbuf_byte_offset=0,
        )
    )


def _bc(ap, dims):
    """Return copy of AP with free dims replaced by `dims` ([step, count])."""
    return bass.AP(ap.tensor, ap.offset, [ap.ap[0]] + dims)


# ---------------------------------------------------------------- host prep
def _wrap(flat):
    w16 = flat.reshape(-1, 16).T.astype(np.int16)       # [16, L/16]
    return np.tile(w16, (8, 1))                         # [128, L/16]


def _build_plan(src, dst):
    """src/dst: RANDOM edges only (no self-loops), int64."""
    E = len(src)
    stream = (src >= HALF).astype(np.int64)
    key = dst * 2 + stream
    order = np.argsort(key, kind="stable")
    s_src = src[order]
    s_dst = dst[order]
    s_str = stream[order]
    s_key = key[order]
    run_start = np.searchsorted(s_key, np.arange(NPAD * 2), side="left")
    rank = np.arange(E) - run_start[s_key]

    s0 = np.where(s_str == 0, S0A, S0B)
    dense = rank < s0
    w = s_dst >> 7
    wl = w % WPC
    core = w // WPC
    g = wl // GW
    wg = wl % GW
    p = s_dst & 127

    # ---- dense index arrays (per core, group-major flat)
    iaA = np.full((NCORES, WPC * S0A * 128), -1, np.int64)
    iaB = np.full((NCORES, WPC * S0B * 128), -1, np.int64)
    dA = dense & (s_str == 0)
    posA = ((g[dA] * GW + wg[dA]) * S0A + rank[dA]) * 128 + p[dA]
    iaA[core[dA], posA] = s_src[dA]
    dB = dense & (s_str == 1)
    posB = ((g[dB] * GW + wg[dB]) * S0B + rank[dB]) * 128 + p[dB]
    iaB[core[dB], posB] = s_src[dB] - HALF

    # ---- overflow, static per-(group, window, stream) block counts
    ovmask = ~dense
    novr = np.zeros((NCORES, NGRP, GW, 2), np.int64)   # overflow edge counts
    np.add.at(novr, (core[ovmask], g[ovmask], wg[ovmask], s_str[ovmask]), 1)
    nblk = -(-novr.max(axis=0) // 128)                  # [NGRP, GW, 2] static
    # block-column layout per group: [w0.A .. w6.A | w0.B .. w6.B]
    colA = np.zeros((NGRP, GW), np.int64)
    colB = np.zeros((NGRP, GW), np.int64)
    goff = np.zeros(NGRP + 1, np.int64)
    for gg in range(NGRP):
        off = goff[gg]
        for ww in range(GW):
            colA[gg, ww] = off
            off += nblk[gg, ww, 0]
        for ww in range(GW):
            colB[gg, ww] = off
            off += nblk[gg, ww, 1]
        goff[gg + 1] = off
    totov = int(goff[-1])
    novAg = [int(nblk[gg, :, 0].sum()) for gg in range(NGRP)]
    novBg = [int(nblk[gg, :, 1].sum()) for gg in range(NGRP)]

    # per-edge overflow placement: position within the (core, window, stream)
    # overflow list decides (block, row)
    colbase = np.where(s_str == 0, colA[g, wg], colB[g, wg])
    okey = (w * 2 + s_str)                              # (window, stream) id
    om = np.nonzero(ovmask)[0]
    oord = om[np.argsort(okey[om], kind="stable")]
    ok = okey[oord]
    ostart = np.searchsorted(ok, np.arange(NPAD // 128 * 2 + 1), side="left")
    opos = np.arange(len(oord)) - ostart[ok]
    ocol = np.zeros(E, np.int64)
    orow = np.zeros(E, np.int64)
    ocol[oord] = colbase[oord] + (opos >> 7)
    orow[oord] = opos & 127

    # Gather idx arrays must contain no mid-stream negatives (HW ring
    # bookkeeping counts valid idxs).  Padded slots gather row 0 and a
    # {0,1} mask zeroes their ex afterwards.
    metas = []
    oidx = om
    SDm = S0A + S0B
    for c in range(NCORES):
        m = oidx[core[oidx] == c]
        io = np.full(totov * 128, -1, np.int64)
        iad = np.full(totov * 128, -1, np.int64)
        dlm = np.full((128, totov), -1.0, np.float16)
        pos = ocol[m] * 128 + orow[m]
        io[pos] = s_src[m] - s_str[m] * HALF
        iad[pos] = wl[m] * 128 + p[m]
        dlm[orow[m], ocol[m]] = p[m].astype(np.float16)
        # dense mask: [128, WPC*SDm] (group-major: [A blocks | B blocks])
        maskd = np.zeros((128, WPC * SDm), np.float16)
        ia_c, ib_c = iaA[c], iaB[c]
        mA = (ia_c >= 0).reshape(NGRP, GW, S0A, 128)
        mB = (ib_c >= 0).reshape(NGRP, GW, S0B, 128)
        for gg in range(NGRP):
            c0 = gg * GW * SDm
            maskd[:, c0:c0 + GW * S0A] = (
                mA[gg].reshape(GW * S0A, 128).T.astype(np.float16))
            maskd[:, c0 + GW * S0A:c0 + GW * SDm] = (
                mB[gg].reshape(GW * S0B, 128).T.astype(np.float16))
        masko = (io.reshape(totov, 128) >= 0).T.astype(np.float16)
        metas.append(dict(
            meta_ia=_wrap(np.maximum(ia_c, 0)),
            meta_ib=_wrap(np.maximum(ib_c, 0)),
            meta_io=_wrap(np.maximum(io, 0)),
            meta_iad=_wrap(np.maximum(iad, 0)),
            meta_dl=dlm, meta_maskd=maskd, meta_masko=masko))

    wmap = []   # [g][w] -> list of block cols (group-relative), A then B
    for gg in range(NGRP):
        rows = []
        for ww in range(GW):
            cols = [int(colA[gg, ww] - goff[gg]) + b
                    for b in range(int(nblk[gg, ww, 0]))]
            cols += [int(colB[gg, ww] - goff[gg]) + b
                     for b in range(int(nblk[gg, ww, 1]))]
            rows.append(cols)
        wmap.append(rows)
    plan = dict(totov=totov, novAg=novAg, novBg=novBg,
                goff=[int(x) for x in goff], wmap=wmap)
    return plan, metas


def _pack_weights(W1, as1, ad1, b1, W2, as2, ad2, b2):
    """Host packing with (c-major, head-minor) column interleave for layer 1."""
    H, CH = as1.shape  # 4, 32
    perm = np.array([hd * CH + c for c in range(CH) for hd in range(H)])
    W1p = W1[:, perm]                                   # [128, 128]
    As1 = np.zeros((128, H), np.float64)
    Ad1 = np.zeros((128, H), np.float64)
    for c in range(CH):
        for hd in range(H):
            As1[c * H + hd, hd] = as1[hd, c]
            Ad1[c * H + hd, hd] = ad1[hd, c]
    W1cat = np.concatenate([W1p, W1p @ As1, W1p @ Ad1], 1
                           ).astype(np.float16)   # [128,136]
    W2p = W2[perm, :]                                   # [128, 64]
    As2 = W2p @ as2[0]
    Ad2 = W2p @ ad2[0]
    W2cat = np.concatenate([W2p, As2[:, None], Ad2[:, None]], 1
                           ).astype(np.float16)          # [128, 66]
    b1rep = np.tile(b1[perm].astype(np.float32), (128, 1))   # [128,128]
    b2rep = np.tile(b2.astype(np.float32), (128, 1))         # [128, 64]
    return W1cat, W2cat, b1rep, b2rep


# ---------------------------------------------------------------- program
def _build_program(plan):
    nc = bacc.Bacc(None, target_bir_lowering=False, num_swdge_queues=NQ)
    totov = plan["totov"]
    goff = plan["goff"]
    wmap = plan["wmap"]
    novAg, novBg = plan["novAg"], plan["novBg"]
    lenA16 = WPC * S0A * 8
    lenB16 = WPC * S0B * 8
    lo16 = totov * 8

    xTm = nc.declare_dram_parameter("xTm", [128, SLICE], F16, isOutput=False)
    W1cat = nc.declare_dram_parameter("W1cat", [128, 136], F16, isOutput=False)
    W2cat = nc.declare_dram_parameter("W2cat", [128, 66], F16, isOutput=False)
    b1rep = nc.declare_dram_parameter("b1rep", [128, 128], F32, isOutput=False)
    b2rep = nc.declare_dram_parameter("b2rep", [128, 64], F32, isOutput=False)
    iota = nc.declare_dram_parameter("iota", [128, 128], F16, isOutput=False)
    meta_ia = nc.declare_dram_parameter("meta_ia", [128, lenA16], I16,
                                        isOutput=False)
    meta_ib = nc.declare_dram_parameter("meta_ib", [128, lenB16], I16,
                                        isOutput=False)
    meta_io = nc.declare_dram_parameter("meta_io", [128, lo16], I16,
                                        isOutput=False)
    meta_iad = nc.declare_dram_parameter("meta_iad", [128, lo16], I16,
                                         isOutput=False)
    meta_dl = nc.declare_dram_parameter("meta_dl", [128, totov], F16,
                                        isOutput=False)
    meta_maskd = nc.declare_dram_parameter(
        "meta_maskd", [128, WPC * (S0A + S0B)], F16, isOutput=False)
    meta_masko = nc.declare_dram_parameter(
        "meta_masko", [128, totov], F16, isOutput=False)
    out = nc.declare_dram_parameter("out", [SLICE, 64], F32, isOutput=True)

    t1own = nc.dram_tensor("t1own", [SLICE, 256], F16)
    adloc = nc.dram_tensor("adloc", [SLICE, 128], F16)
    table1 = nc.dram_tensor("table1", [NPAD, 256], F16, addr_space="Shared")
    h1 = nc.dram_tensor("h1", [SLICE, 128], F16)
    t2own = nc.dram_tensor("t2own", [SLICE, 128], F16)
    t2full = nc.dram_tensor("t2full", [NPAD, 128], F16, addr_space="Shared")

    with tile.TileContext(nc) as tc:
        with (
            tc.tile_pool(name="const", bufs=1) as cp,
            tc.tile_pool(name="meta", bufs=1) as mp,
            tc.tile_pool(name="proj", bufs=3) as pp,
            tc.tile_pool(name="projps", bufs=3, space="PSUM") as pps,
            tc.tile_pool(name="gd", bufs=2) as gdp,
            tc.tile_pool(name="gov", bufs=2) as gop,
            tc.tile_pool(name="oh", bufs=2) as ohp,
            tc.tile_pool(name="tt", bufs=2) as ttp,
            tc.tile_pool(name="rr", bufs=2) as rrp,
            tc.tile_pool(name="edgeps", bufs=4, space="PSUM") as eps,
            tc.tile_pool(name="epi", bufs=2) as qp,
        ):
            # ---- persistent constants / metadata
            w1_sb = cp.tile([128, 136], F16)
            nc.sync.dma_start(out=w1_sb[:], in_=W1cat[:])
            w2_sb = cp.tile([128, 66], F16)
            nc.sync.dma_start(out=w2_sb[:], in_=W2cat[:])
            b1_sb = cp.tile([128, 128], F32)
            nc.sync.dma_start(out=b1_sb[:], in_=b1rep[:])
            b2_sb = cp.tile([128, 64], F32)
            nc.sync.dma_start(out=b2_sb[:], in_=b2rep[:])
            iota_sb = cp.tile([128, 128], F16)
            nc.sync.dma_start(out=iota_sb[:], in_=iota[:])
            ia_sb = mp.tile([128, lenA16], I16)
            nc.scalar.dma_start(out=ia_sb[:], in_=meta_ia[:])
            ib_sb = mp.tile([128, lenB16], I16)
            nc.scalar.dma_start(out=ib_sb[:], in_=meta_ib[:])
            io_sb = mp.tile([128, lo16], I16)
            nc.scalar.dma_start(out=io_sb[:], in_=meta_io[:])
            iad_sb = mp.tile([128, lo16], I16)
            nc.scalar.dma_start(out=iad_sb[:], in_=meta_iad[:])
            dl_sb = mp.tile([128, totov], F16)
            nc.scalar.dma_start(out=dl_sb[:], in_=meta_dl[:])
            mkd_sb = mp.tile([128, WPC * (S0A + S0B)], F16)
            nc.scalar.dma_start(out=mkd_sb[:], in_=meta_maskd[:])
            mko_sb = mp.tile([128, totov], F16)
            nc.scalar.dma_start(out=mko_sb[:], in_=meta_masko[:])

            # ---- P1: own-slice projection -> t1own (+ adloc)
            for s in range(NGRP):
                xt = pp.tile([128, 128 * GW], F16, tag="xt")
                nc.sync.dma_start(
                    out=xt[:], in_=xTm[:, s * 128 * GW:(s + 1) * 128 * GW])
                rows = pp.tile([128, GW, 136], F16, tag="rows")
                for j in range(GW):
                    ps = pps.tile([128, 136], F32, tag="pp")
                    nc.tensor.matmul(out=ps[:], lhsT=xt[:, j * 128:(j + 1) * 128],
                                     rhs=w1_sb[:], start=True, stop=True)
                    if j % 2 == 0:
                        nc.scalar.copy(out=rows[:, j, :], in_=ps[:])
                    else:
                        nc.vector.tensor_copy(out=rows[:, j, :], in_=ps[:])
                r0 = s * 128 * GW
                dst = t1own[r0:r0 + 128 * GW, 0:136]
                dst = dst.rearrange("(j p) c -> p j c", p=128)
                nc.scalar.dma_start(out=dst, in_=rows[:, :, :])
                dst2 = adloc[r0:r0 + 128 * GW, 0:4]
                dst2 = dst2.rearrange("(j p) c -> p j c", p=128)
                nc.sync.dma_start(out=dst2, in_=rows[:, :, 132:136])

            tc.strict_bb_all_engine_barrier()
            nc.gpsimd.collective_compute(
                "AllGather", OP.bypass,
                replica_groups=[list(range(NCORES))],
                ins=[t1own[:]], outs=[table1[:]])
            tc.strict_bb_all_engine_barrier()

            # ---- edge phase (both layers)
            def nextq():
                return 0

            def edge_phase(layer):
                if layer == 1:
                    elem, adw, hc = 132, 4, 128
                    tblA = table1[0:HALF, 0:elem]
                    tblB = table1[HALF:NPAD, 0:elem]
                    adgt = adloc[:, 0:4]
                    ownt, oc0, oc1, ac0, ac1 = t1own, 0, 132, 132, 136
                    estep = 256
                else:
                    elem, adw, hc = 66, 1, 64
                    tblA = t2full[0:HALF, 0:elem]
                    tblB = t2full[HALF:NPAD, 0:elem]
                    adgt = t2own[:, 65:66]
                    ownt, oc0, oc1, ac0, ac1 = t2own, 0, 66, 65, 66
                    estep = 128
                rw = hc + adw

                def front(g):
                    novA, novB = novAg[g], novBg[g]
                    nov = novA + novB
                    ovo = goff[g]
                    g0 = g * GW * 128
                    Gd = gdp.tile([128, NBD, elem], F16, tag="gd")
                    selfsrc = ownt[g0:g0 + GW * 128, oc0:oc1]
                    selfsrc = selfsrc.rearrange("(j p) c -> p j c", p=128)
                    nc.sync.dma_start(out=Gd[:, 0:GW, :], in_=selfsrc)
                    ADW = ttp.tile([128, GW, adw], F16, tag="adw")
                    adsrc = ownt[g0:g0 + GW * 128, ac0:ac1]
                    adsrc = adsrc.rearrange("(j p) c -> p j c", p=128)
                    nc.sync.dma_start(out=ADW[:], in_=adsrc)
                    _dma_gather_raw(
                        nc, Gd[:, GW:GW + GW * S0A, :], tblA,
                        ia_sb[:, g * GW * S0A * 8:(g + 1) * GW * S0A * 8],
                        GW * S0A * 128, elem, estep, nextq())
                    _dma_gather_raw(
                        nc, Gd[:, GW + GW * S0A:NBD, :], tblB,
                        ib_sb[:, g * GW * S0B * 8:(g + 1) * GW * S0B * 8],
                        GW * S0B * 128, elem, estep, nextq())
                    st = dict(Gd=Gd, ADW=ADW, nov=nov, novA=novA, ovo=ovo)
                    if nov:
                        Gov = gop.tile([128, nov, elem], F16, tag="go")
                        ADO = ttp.tile([128, nov, adw], F16, tag="ado")
                        if novA:
                            _dma_gather_raw(
                                nc, Gov[:, 0:novA, :], tblA,
                                io_sb[:, ovo * 8:(ovo + novA) * 8],
                                novA * 128, elem, estep, nextq())
                        if novB:
                            _dma_gather_raw(
                                nc, Gov[:, novA:nov, :], tblB,
                                io_sb[:, (ovo + novA) * 8:(ovo + nov) * 8],
                                novB * 128, elem, estep, nextq())
                        _dma_gather_raw(
                            nc, ADO[:, :, :], adgt,
                            iad_sb[:, ovo * 8:(ovo + nov) * 8],
                            nov * 128, adw, 128, nextq())
                        OHt = ohp.tile([128, nov, 128], F16, tag="oh")
                        nc.vector.tensor_tensor(
                            out=OHt[:, :, :],
                            in0=_bc(iota_sb[:, 0:1], [[0, nov], [1, 128]]),
                            in1=_bc(dl_sb[:, ovo:ovo + 1], [[1, nov], [0, 128]]),
                            op=OP.is_equal)
                        st.update(Gov=Gov, ADO=ADO, OHt=OHt)
                    return st

                def back(g, st):
                    nov, novA, ovo = st["nov"], st["novA"], st["ovo"]
                    Gd, ADW = st["Gd"], st["ADW"]
                    g0 = g * GW * 128
                    a0, a1 = GW, GW + GW * S0A
                    b0, b1c = a1, NBD
                    # ---- dense ex = exp(leaky(a_s + a_d))
                    T = ttp.tile([128, NBD, adw], F16, tag="T")
                    nc.vector.tensor_tensor(
                        out=T[:, 0:GW, :], in0=Gd[:, 0:GW, hc:hc + adw],
                        in1=ADW[:, :, :], op=OP.add)
                    nc.vector.tensor_tensor(
                        out=T[:, a0:a1, :].rearrange("q (w s) c -> q w s c",
                                                     s=S0A),
                        in0=Gd[:, a0:a1, hc:hc + adw].rearrange(
                            "q (w s) c -> q w s c", s=S0A),
                        in1=_bc(ADW[:, 0:1, :], [[adw, GW], [0, S0A], [1, adw]]),
                        op=OP.add)
                    nc.vector.tensor_tensor(
                        out=T[:, b0:b1c, :].rearrange("q (w s) c -> q w s c",
                                                      s=S0B),
                        in0=Gd[:, b0:b1c, hc:hc + adw].rearrange(
                            "q (w s) c -> q w s c", s=S0B),
                        in1=_bc(ADW[:, 0:1, :], [[adw, GW], [0, S0B], [1, adw]]),
                        op=OP.add)
                    T2 = ttp.tile([128, NBD, adw], F16, tag="T2")
                    nc.vector.tensor_scalar(
                        out=T2[:], in0=T[:], scalar1=NEG_SLOPE, scalar2=None,
                        op0=OP.mult)
                    nc.vector.tensor_tensor(out=T[:], in0=T[:], in1=T2[:],
                                            op=OP.max)
                    nc.scalar.activation(out=Gd[:, :, hc:hc + adw], in_=T[:],
                                         func=AF.Exp)
                    nc.vector.tensor_tensor(
                        out=Gd[:, GW:NBD, hc:hc + adw],
                        in0=Gd[:, GW:NBD, hc:hc + adw],
                        in1=_bc(mkd_sb[:, g * GW * (S0A + S0B):
                                       g * GW * (S0A + S0B) + 1],
                                [[1, GW * (S0A + S0B)], [0, adw]]),
                        op=OP.mult)
                    # ---- h *= ex (broadcast)
                    if layer == 1:
                        hview = Gd[:, :, 0:hc].rearrange(
                            "q b (c h) -> q b c h", h=4)
                        exb = _bc(Gd[:, 0:1, hc:hc + adw],
                                  [[elem, NBD], [0, 32], [1, 4]])
                    else:
                        hview = Gd[:, :, 0:hc]
                        exb = _bc(Gd[:, 0:1, hc:hc + adw],
                                  [[elem, NBD], [0, hc]])
                    nc.vector.tensor_tensor(out=hview, in0=hview, in1=exb,
                                            op=OP.mult)
                    # ---- reduce dense regions -> RS [128, GW, rw] f32
                    RS = rrp.tile([128, GW, rw], F32, tag="RS")
                    nc.scalar.copy(out=RS[:], in_=Gd[:, 0:GW, 0:rw])
                    RA = rrp.tile([128, GW, rw], F32, tag="RA")
                    nc.vector.tensor_reduce(
                        out=RA[:],
                        in_=Gd[:, a0:a1, 0:rw].rearrange(
                            "q (w s) c -> q w c s", s=S0A),
                        axis=mybir.AxisListType.X, op=OP.add)
                    RB = rrp.tile([128, GW, rw], F32, tag="RB")
                    nc.vector.tensor_reduce(
                        out=RB[:],
                        in_=Gd[:, b0:b1c, 0:rw].rearrange(
                            "q (w s) c -> q w c s", s=S0B),
                        axis=mybir.AxisListType.X, op=OP.add)
                    nc.vector.tensor_tensor(out=RS[:], in0=RS[:], in1=RA[:],
                                            op=OP.add)
                    nc.vector.tensor_tensor(out=RS[:], in0=RS[:], in1=RB[:],
                                            op=OP.add)
                    # ---- overflow: ex chain + scatter matmuls
                    if nov:
                        Gov, ADO, OHt = st["Gov"], st["ADO"], st["OHt"]
                        To = ttp.tile([128, nov, adw], F16, tag="To")
                        nc.vector.tensor_tensor(
                            out=To[:], in0=Gov[:, :, hc:hc + adw],
                            in1=ADO[:], op=OP.add)
                        To2 = ttp.tile([128, nov, adw], F16, tag="To2")
                        nc.vector.tensor_scalar(
                            out=To2[:], in0=To[:], scalar1=NEG_SLOPE,
                            scalar2=None, op0=OP.mult)
                        nc.vector.tensor_tensor(out=To[:], in0=To[:],
                                                in1=To2[:], op=OP.max)
                        nc.scalar.activation(out=Gov[:, :, hc:hc + adw],
                                             in_=To[:], func=AF.Exp)
                        nc.vector.tensor_tensor(
                            out=Gov[:, :, hc:hc + adw],
                            in0=Gov[:, :, hc:hc + adw],
                            in1=_bc(mko_sb[:, ovo:ovo + 1],
                                    [[1, nov], [0, adw]]),
                            op=OP.mult)
                        if layer == 1:
                            hv = Gov[:, :, 0:hc].rearrange(
                                "q b (c h) -> q b c h", h=4)
                            eb = _bc(Gov[:, 0:1, hc:hc + adw],
                                     [[elem, nov], [0, 32], [1, 4]])
                        else:
                            hv = Gov[:, :, 0:hc]
                            eb = _bc(Gov[:, 0:1, hc:hc + adw],
                                     [[elem, nov], [0, hc]])
                        nc.vector.tensor_tensor(out=hv, in0=hv, in1=eb,
                                                op=OP.mult)
                        for w in range(GW):
                            cols = wmap[g][w]
                            if not cols:
                                continue
                            ps = eps.tile([128, rw], F32, tag="eps")
                            for ji, j in enumerate(cols):
                                nc.tensor.matmul(
                                    out=ps[:, 0:rw], lhsT=OHt[:, j, :],
                                    rhs=Gov[:, j, 0:rw],
                                    start=(ji == 0), stop=(ji == len(cols) - 1))
                            nc.vector.tensor_tensor(
                                out=RS[:, w, :], in0=RS[:, w, :],
                                in1=ps[:, 0:rw], op=OP.add)
                    # ---- epilogue (batched over the group)
                    rc = qp.tile([128, GW, adw], F32, tag="rc")
                    nc.vector.reciprocal(rc[:], RS[:, :, hc:hc + adw])
                    z = qp.tile([128, GW, hc], F32, tag="z")
                    if layer == 1:
                        zv = z[:].rearrange("q w (c h) -> q w c h", h=4)
                        nv = RS[:, :, 0:hc].rearrange("q w (c h) -> q w c h",
                                                      h=4)
                        rcb = _bc(rc[:, 0:1, :], [[adw, GW], [0, 32], [1, 4]])
                    else:
                        zv = z[:]
                        nv = RS[:, :, 0:hc]
                        rcb = _bc(rc[:, 0:1, :], [[adw, GW], [0, hc]])
                    nc.vector.tensor_tensor(out=zv, in0=nv, in1=rcb,
                                            op=OP.mult)
                    bias = b1_sb if layer == 1 else b2_sb
                    nc.vector.tensor_tensor(
                        out=z[:], in0=z[:],
                        in1=_bc(bias[:, 0:1], [[0, GW], [1, hc]]),
                        op=OP.add)
                    if layer == 1:
                        m = qp.tile([128, GW, hc], F32, tag="m")
                        nc.vector.tensor_scalar(
                            out=m[:], in0=z[:], scalar1=0.0, scalar2=None,
                            op0=OP.min)
                        e = qp.tile([128, GW, hc], F32, tag="e")
                        nc.scalar.activation(out=e[:], in_=m[:], func=AF.Exp)
                        r = qp.tile([128, GW, hc], F32, tag="r")
                        nc.vector.tensor_scalar(
                            out=r[:], in0=z[:], scalar1=0.0, scalar2=-1.0,
                            op0=OP.max, op1=OP.add)
                        hp = qp.tile([128, GW, hc], F16, tag="hp")
                        nc.vector.tensor_tensor(out=hp[:], in0=e[:], in1=r[:],
                                                op=OP.add)
                        dsth = h1[g0:g0 + GW * 128, :]
                        dsth = dsth.rearrange("(j p) c -> p j c", p=128)
                        nc.sync.dma_start(out=dsth, in_=hp[:])
                    else:
                        dsto = out[g0:g0 + GW * 128, :]
                        dsto = dsto.rearrange("(j p) c -> p j c", p=128)
                        nc.scalar.dma_start(out=dsto, in_=z[:])

                st = None
                for gi in range(NGRP + 1):
                    nst = front(gi) if gi < NGRP else None
                    if gi >= 1:
                        back(gi - 1, st)
                    st = nst

            edge_phase(1)
            tc.strict_bb_all_engine_barrier()

            # ---- P2: own-slice projection -> t2own
            for s2 in range(NGRP):
                lt = pp.tile([128, 128 * GW], F16, tag="lt")
                nc.sync.dma_start_transpose(
                    out=lt[:],
                    in_=h1[s2 * GW * 128:(s2 + 1) * GW * 128, :])
                rows = pp.tile([128, GW, 66], F16, tag="rows2")
                for j in range(GW):
                    ps = pps.tile([128, 136], F32, tag="pp")
                    nc.tensor.matmul(out=ps[:, 0:66],
                                     lhsT=lt[:, j * 128:(j + 1) * 128],
                                     rhs=w2_sb[:], start=True, stop=True)
                    if j % 2 == 0:
                        nc.scalar.copy(out=rows[:, j, :], in_=ps[:, 0:66])
                    else:
                        nc.vector.tensor_copy(out=rows[:, j, :], in_=ps[:, 0:66])
                dst = t2own[s2 * GW * 128:(s2 + 1) * GW * 128, 0:66]
                dst = dst.rearrange("(j p) c -> p j c", p=128)
                nc.sync.dma_start(out=dst, in_=rows[:, :, :])

            tc.strict_bb_all_engine_barrier()
            nc.gpsimd.collective_compute(
                "AllGather", OP.bypass,
                replica_groups=[list(range(NCORES))],
                ins=[t2own[:]], outs=[t2full[:]])
            tc.strict_bb_all_engine_barrier()
            edge_phase(2)

    nc.finalize()
    return nc


# ---------------------------------------------------------------- runner
def _make_runner(nc, n_cores):
    import jax
    from jax.sharding import Mesh, PartitionSpec, NamedSharding
    from jax.experimental.shard_map import shard_map
    from concourse.bass2jax import (_bass_exec_p, partition_id_tensor,
                                    install_neuronx_cc_hook)

    install_neuronx_cc_hook()
    partition_name = nc.partition_id_tensor.name if nc.partition_id_tensor else None
    in_names, out_names, out_avals, zero_outs = [], [], [], []
    for alloc in nc.m.functions[0].allocations:
        if not isinstance(alloc, mybir.MemoryLocationSet):
            continue
        name = alloc.memorylocations[0].name
        if alloc.kind == "ExternalInput":
            if name != partition_name:
                in_names.append(name)
        elif alloc.kind == "ExternalOutput":
            out_names.append(name)
            shape = tuple(alloc.tensor_shape)
            dtype = mybir.dt.np(alloc.dtype)
            out_avals.append(jax.core.ShapedArray(shape, dtype))
            zero_outs.append(np.zeros(shape, dtype))

    n_params = len(in_names)
    n_outs = len(out_avals)
    all_in = list(in_names) + list(out_names)
    if partition_name is not None:
        all_in.append(partition_name)

    def _body(*args):
        operands = list(args)
        if partition_name is not None:
            operands.append(partition_id_tensor())
        outs = _bass_exec_p.bind(
            *operands, out_avals=tuple(out_avals), in_names=tuple(all_in),
            out_names=tuple(out_names), lowering_input_output_aliases=(),
            sim_require_finite=False, sim_require_nnan=False, nc=nc)
        return tuple(outs)

    import os
    if os.environ.get("BASS_SIM_CPU") == "1":
        devices = jax.devices("cpu")[:n_cores]
    else:
        devices = jax.devices()[:n_cores]
    mesh = Mesh(np.asarray(devices), ("core",))
    specs = (PartitionSpec("core"),) * (n_params + n_outs)
    sharded = jax.jit(
        shard_map(_body, mesh=mesh, in_specs=specs,
                  out_specs=(PartitionSpec("core"),) * n_outs, check_rep=False),
        keep_unused=True)

    shard = NamedSharding(mesh, PartitionSpec("core"))
    devcache = {}

    def run(in_maps):
        import hashlib
        hsh = hashlib.md5()
        for m in in_maps:
            for nm in in_names:
                hsh.update(np.asarray(m[nm]).tobytes())
        key = hsh.hexdigest()
        dev = devcache.get(key)
        if dev is None:
            per_core = [[np.asarray(m[nm]) for nm in in_names] for m in in_maps]
            concat_in = [
                np.concatenate([per_core[c][i] for c in range(n_cores)], 0)
                for i in range(n_params)]
            concat_zero = [
                np.zeros((n_cores * z.shape[0], *z.shape[1:]), z.dtype)
                for z in zero_outs]
            dev = ([jax.device_put(a, shard) for a in concat_in],
                   [jax.device_put(z, shard) for z in concat_zero])
            jax.block_until_ready(dev)
            devcache.clear()
            devcache[key] = dev
        dev_in, dev_zero = dev
        outs = sharded(*dev_in, *dev_zero)
        jax.block_until_ready(outs)
        return [
            {nm: np.asarray(outs[i]).reshape(n_cores, *out_avals[i].shape)[c]
             for i, nm in enumerate(out_names)}
            for c in range(n_cores)
        ]

    return run


_CACHE = {}
_LAST_NC = [None]


def last_nc():
    return _LAST_NC[0]


def kernel(x, edge_index, W1, att_src1, att_dst1, b1, W2, att_src2, att_dst2, b2):
    x = np.asarray(x)
    edge_index = np.asarray(edge_index)
    src = edge_index[0].astype(np.int64)
    dst = edge_index[1].astype(np.int64)

    ck = hash((src.tobytes(), dst.tobytes()))
    if ck in _CACHE:
        plan, metas, run = _CACHE[ck]
    else:
        plan, metas = _build_plan(src, dst)
        nc = _build_program(plan)
        run = _make_runner(nc, NCORES)
        _CACHE[ck] = (plan, metas, run)
        _LAST_NC[0] = nc

    W1cat, W2cat, b1rep, b2rep = _pack_weights(
        np.asarray(W1, np.float64), np.asarray(att_src1, np.float64),
        np.asarray(att_dst1, np.float64), np.asarray(b1, np.float64),
        np.asarray(W2, np.float64), np.asarray(att_src2, np.float64),
        np.asarray(att_dst2, np.float64), np.asarray(b2, np.float64))
    xT = np.zeros((128, NPAD), np.float16)
    xT[:, :N] = np.asarray(x, np.float32).T.astype(np.float16)
    iota = np.tile(np.arange(128, dtype=np.float16), (128, 1))

    in_maps = []
    for c in range(NCORES):
        m = dict(metas[c])
        m.update(xTm=np.ascontiguousarray(xT[:, c * SLICE:(c + 1) * SLICE]),
                 W1cat=W1cat, W2cat=W2cat, b1rep=b1rep, b2rep=b2rep, iota=iota)
        in_maps.append(m)

    res = run(in_maps)
    full = np.concatenate([res[c]["out"] for c in range(NCORES)], 0)
    return full[:N].astype(np.float32)
